# revision 1
# baseline (speedup 1.0000x reference)
"""GCN (3-layer GCNConv + mean-pool + MLP head) Trainium2 Bass kernel, 8 NeuronCores.

Strategy (graph/data parallel, per sharding hint):
  - Destination nodes are partitioned into 8 contiguous blocks (one per core);
    each core owns SHARD=12544 destinations = 98 windows of 128.
  - Node features live in DRAM tables with 256B rows ([NPAD, 128] bf16, first
    F_in columns real) so the custom dma_gather instruction (InstDMAGatherAnt)
    can fetch thousands of source rows per instruction instead of one
    indirect DMA per 128 edges (the baseline's bottleneck: ~1us SWDGE fixed
    cost per 128-row indirect DMA). dma_gather needs single_packet=False
    above 1024 idxs (4KB/lane packet limit) and crashes somewhere above
    8192 idxs/instruction; gathers are chunked to <=2560 idxs and spread
    round-robin over the 4 SWDGE queues so their latency-bound drains
    overlap (NOTE: CoreSim mis-attributes dma_gather queue_num and raises a
    sem-lock error in simulation; hardware-verified correct).
  - The node id space is remapped segment-major into 4 regions of <=28672
    rows so gather indices fit int16; edges are bucketed by (piece of 7 dst
    windows, source segment, dst window), each bucket padded to full
    128-edge columns. Buckets use the max column count over cores so the
    SPMD program is identical on every core.
  - Per piece: 4 dma_gathers (one per source segment), one norm multiply,
    then per window a one-hot S matmul chain accumulates messages in PSUM
    (aggregate-then-transform). Self-loops are regular grid edges.
  - Layer outputs are written compact ([12544, F]), AllGathered per segment
    (overlapped with remaining gathers), and expanded on-device to the padded
    gather table for the next layer.
  - Layer-3 output is mean-pooled per graph via one-hot matmuls into a PSUM
    accumulator, AllReduced, and the tiny FC head runs replicated.
"""

import os
import sys
from dataclasses import dataclass

import numpy as np
import ml_dtypes

for _p in ("/opt/trn_rl_repo", "/root/.axon_site/_ro/trn_rl_repo"):
    if os.path.isdir(_p) and _p not in sys.path:
        sys.path.insert(0, _p)

bf16 = ml_dtypes.bfloat16
P = 128
N = 100000
G = 128
F = (40, 40, 80, 160)
HID = 128
NCORES = 8
SHARD = 12544
NW = 98                       # windows per core
CW = 7                        # windows per piece
NP = NW // CW                 # 14 pieces
SEG_W = (28, 28, 28, 14)      # windows per segment
SEG_NODES = tuple(w * P for w in SEG_W)            # 3584,3584,3584,1792
SEG_WSTART = (0, 28, 56, 84)
SEG_START = tuple(w * P for w in SEG_WSTART)       # node offset within shard
REG_SIZE = tuple(NCORES * n for n in SEG_NODES)    # 28672*3, 14336
REG_BASE = (0, 28672, 57344, 86016)
NPAD = NCORES * SHARD         # 100352
PIECE_SEG = tuple(min(p // 4, 3) for p in range(NP))  # piece -> its dst segment
NSEG = 4


# ---------------------------------------------------------------- host prep

def _remap_rows():
    """node id -> segment-major global table row."""
    v = np.arange(NPAD, dtype=np.int64)
    c, r = v // SHARD, v % SHARD
    s = np.minimum(r // SEG_NODES[0], 3)
    row = (np.asarray(REG_BASE)[s] + c * np.asarray(SEG_NODES)[s]
           + (r - np.asarray(SEG_START)[s]))
    return row


@dataclass
class Structure:
    ncol: np.ndarray      # [NW, NSEG] columns per (window, seg) bucket
    totcol: int
    totslot: int
    cmax: int             # max columns in a piece
    piece_col0: list      # per piece: first global column
    piece_ncol: list      # per piece: total columns
    gath: list            # per piece: list over seg of (local col off, ncols)
    wruns: list           # per (piece, wi): list of (local col off, ncols, gcol0)

    def key(self):
        return (self.ncol.tobytes(), self.totcol, self.cmax)


def build_structure(ncol):
    """Static (SPMD-uniform) grid layout from the per-(window,seg) column
    counts. Column order: piece-major, then seg, then window."""
    piece_col0, piece_ncol, gath, wruns = [], [], [], []
    col = 0
    for p in range(NP):
        piece_col0.append(col)
        pg = []
        local = 0
        runs = {wi: [] for wi in range(CW)}
        for s in range(NSEG):
            nc_s = 0
            for wi in range(CW):
                w = p * CW + wi
                n = int(ncol[w, s])
                if n:
                    runs[wi].append((local + nc_s, n, col + local + nc_s))
                nc_s += n
            pg.append((local, nc_s))
            local += nc_s
        piece_ncol.append(local)
        gath.append(pg)
        for wi in range(CW):
            wruns.append(runs[wi])
        col += local
    totcol = col
    return Structure(ncol=ncol, totcol=totcol, totslot=totcol * P,
                     cmax=max(piece_ncol), piece_col0=piece_col0,
                     piece_ncol=piece_ncol, gath=gath, wruns=wruns)


def build_host_data(inp):
    src = np.asarray(inp["edge_index"][0]).astype(np.int64).ravel()
    dst = np.asarray(inp["edge_index"][1]).astype(np.int64).ravel()
    batch = np.asarray(inp["batch"]).astype(np.int64).ravel()
    deg = (np.bincount(dst, minlength=N) + 1).astype(np.float64)
    dis = 1.0 / np.sqrt(deg)
    # self-loops folded in as regular edges with norm 1/deg
    loop = np.arange(N, dtype=np.int64)
    srcA = np.concatenate([src, loop])
    dstA = np.concatenate([dst, loop])
    norm = np.concatenate([dis[src] * dis[dst], 1.0 / deg]).astype(np.float32)

    remap = _remap_rows()
    srow = remap[srcA]
    seg = np.minimum(srow // REG_SIZE[0], 3)
    lidx = srow - np.asarray(REG_BASE)[seg]            # int16-safe (<28672)

    core = dstA // SHARD
    r = dstA % SHARD
    w = r // P
    dloc = r % P
    p = w // CW

    # per-core per-(w,seg) counts -> uniform column counts (max over cores)
    kid_full = ((core * NW + w) * NSEG + seg)
    cnt = np.bincount(kid_full, minlength=NCORES * NW * NSEG).reshape(
        NCORES, NW, NSEG)
    ncol = np.ceil(cnt.max(axis=0) / P).astype(np.int64)   # [NW, NSEG]
    ncol = np.maximum(ncol, 1)
    st = build_structure(ncol)

    # bucket start slots in (piece, seg, window) order
    # bucket id in layout order: for piece p, seg s, window w
    bstart = np.zeros((NW, NSEG), np.int64)
    colc = 0
    for pp in range(NP):
        for s in range(NSEG):
            for wi in range(CW):
                ww = pp * CW + wi
                bstart[ww, s] = colc * P
                colc += int(ncol[ww, s])
    assert colc == st.totcol

    cores = []
    for c in range(NCORES):
        m = core == c
        sl, nm, dl, ww, ss = lidx[m], norm[m], dloc[m], w[m], seg[m]
        # sort by (piece, seg, window) = layout order
        okey = (ww // CW) * (NSEG * NW) + ss * NW + ww
        order = np.argsort(okey, kind="stable")
        sl, nm, dl, ww, ss = (a[order] for a in (sl, nm, dl, ww, ss))
        bid = ww * NSEG + ss
        cnts = np.bincount(bid, minlength=NW * NSEG)
        # j within bucket
        sk = np.argsort(bid, kind="stable")
        inv = np.empty_like(sk)
        inv[sk] = np.arange(len(sk))
        csum = np.concatenate([[0], np.cumsum(cnts)])
        j = inv - csum[bid]
        slot = bstart[ww, ss] + j
        eidx = np.zeros(st.totslot, np.int16)
        enrm = np.zeros(st.totslot, np.float32)
        edl = np.full(st.totslot, -1.0, np.float32)
        eidx[slot] = sl.astype(np.int16)
        enrm[slot] = nm
        edl[slot] = dl.astype(np.float32)
        # idx wrap: slot i -> [i%16, i//16], replicated over 8 groups of 16
        idx16 = eidx.reshape(-1, 16).T.copy()          # [16, totslot/16]
        idx_full = np.tile(idx16, (8, 1))
        # grid wrap: slot i -> [i%128, i//128]
        enrm_g = np.ascontiguousarray(
            enrm.reshape(-1, P).T).astype(bf16)        # [128, totcol]
        edl_g = np.ascontiguousarray(
            edl.reshape(-1, P).T).astype(bf16)
        nid = np.arange(SHARD) + c * SHARD
        gl = np.where(nid < N, batch[np.minimum(nid, N - 1)], -1).astype(np.float32)
        gloc = np.ascontiguousarray(gl.reshape(NW, P).T).astype(bf16)
        cores.append(dict(eidx=idx_full, enrm=enrm_g, edl=edl_g, gloc=gloc))

    # compact x table in remapped row order
    xtc = np.zeros((NPAD, F[0]), bf16)
    xtc[remap[:N]] = np.asarray(inp["x"]).astype(bf16)

    cnt_g = np.bincount(batch, minlength=G).astype(np.float32)
    invc = np.zeros((P, 1), np.float32)
    invc[:G, 0] = 1.0 / np.maximum(cnt_g, 1.0)

    iota = np.tile(np.arange(P, dtype=np.float32), (P, 1)).astype(bf16)
    ident = np.eye(P, dtype=np.float32).astype(bf16)

    def a2(x, dt):
        return np.ascontiguousarray(np.asarray(x), dtype=dt)

    wts = dict(
        w1a=np.concatenate([a2(inp["W1"], bf16), a2(inp["b1"], bf16)[None]], 0),
        w2a=np.concatenate([a2(inp["W2"], bf16), a2(inp["b2"], bf16)[None]], 0),
        w3a=np.concatenate([a2(inp["W3"], bf16), a2(inp["b3"], bf16)[None]], 0),
        fw1=a2(inp["fW1"], bf16),
        fb1c=a2(inp["fb1"], np.float32).reshape(-1, 1),
        fw2=a2(inp["fW2"], bf16),
        invc=invc,
        iota=iota,
        ident=ident,
        xtc=xtc,
    )
    fb2 = float(np.asarray(inp["fb2"]).ravel()[0])
    return st, cores, wts, fb2


# ---------------------------------------------------------------- bass build

def build_bass(st, fb2):
    import concourse.bacc as bacc
    import concourse.bass as bass
    import concourse.mybir as mybir
    import concourse.tile as tile

    dt = mybir.dt
    AF = mybir.ActivationFunctionType
    OP = mybir.AluOpType
    F0, F1, F2, F3 = F
    FMAX = max(F0, F1, F2)
    TMAX = int(st.ncol.sum(axis=1).max())    # max columns per window

    nc = bacc.Bacc("TRN2", target_bir_lowering=False, debug=False,
                   enable_asserts=False, num_devices=NCORES,
                   num_swdge_queues=4)

    # ---- I/O
    xtc_d = nc.dram_tensor("xtc", [NPAD, F0], dt.bfloat16, kind="ExternalInput")
    eidx_d = nc.dram_tensor("eidx", [P, st.totslot // 16], dt.int16,
                            kind="ExternalInput")
    enrm_d = nc.dram_tensor("enrm", [P, st.totcol], dt.bfloat16,
                            kind="ExternalInput")
    edl_d = nc.dram_tensor("edl", [P, st.totcol], dt.bfloat16,
                           kind="ExternalInput")
    gloc_d = nc.dram_tensor("gloc", [P, NW], dt.bfloat16, kind="ExternalInput")
    w1a_d = nc.dram_tensor("w1a", [F0 + 1, F1], dt.bfloat16, kind="ExternalInput")
    w2a_d = nc.dram_tensor("w2a", [F1 + 1, F2], dt.bfloat16, kind="ExternalInput")
    w3a_d = nc.dram_tensor("w3a", [F2 + 1, F3], dt.bfloat16, kind="ExternalInput")
    fw1_d = nc.dram_tensor("fw1", [F3, HID], dt.bfloat16, kind="ExternalInput")
    fb1_d = nc.dram_tensor("fb1c", [HID, 1], dt.float32, kind="ExternalInput")
    fw2_d = nc.dram_tensor("fw2", [HID, 1], dt.bfloat16, kind="ExternalInput")
    invc_d = nc.dram_tensor("invc", [P, 1], dt.float32, kind="ExternalInput")
    iota_d = nc.dram_tensor("iota", [P, P], dt.bfloat16, kind="ExternalInput")
    ident_d = nc.dram_tensor("ident", [P, P], dt.bfloat16, kind="ExternalInput")
    out_d = nc.dram_tensor("out", [1, P], dt.float32, kind="ExternalOutput")
    pdbg_d = nc.dram_tensor("pooled_dbg", [P, F3], dt.float32,
                            kind="ExternalOutput")

    rg = [list(range(NCORES))]

    with tile.TileContext(nc) as tc:
        with (
            tc.tile_pool(name="res", bufs=1) as res,
            tc.tile_pool(name="msgs", bufs=4) as msgsp,
            tc.tile_pool(name="sp", bufs=3) as sp,
            tc.tile_pool(name="work", bufs=3) as work,
            tc.tile_pool(name="hw", bufs=3) as hwp,
            tc.tile_pool(name="pa_ps", bufs=2, space="PSUM") as pa_ps,
            tc.tile_pool(name="p2_ps", bufs=2, space="PSUM") as p2_ps,
            tc.tile_pool(name="pool_ps", bufs=1, space="PSUM") as pool_ps,
            tc.tile_pool(name="head_ps", bufs=1, space="PSUM") as head_ps,
            tc.tile_pool(name="dram", bufs=1, space="DRAM") as dram,
        ):
            # ---- persistent SBUF state
            eidx = res.tile([P, st.totslot // 16], dt.int16)
            enrm = res.tile([P, st.totcol], dt.bfloat16)
            edl = res.tile([P, st.totcol], dt.bfloat16)
            gloc = res.tile([P, NW], dt.bfloat16)
            w1a = res.tile([F0 + 1, F1], dt.bfloat16)
            w2a = res.tile([F1 + 1, F2], dt.bfloat16)
            w3a = res.tile([F2 + 1, F3], dt.bfloat16)
            fw1a = res.tile([F3 // 2, HID], dt.bfloat16)
            fw1b = res.tile([F3 // 2, HID], dt.bfloat16)
            fb1c = res.tile([HID, 1], dt.float32)
            fw2 = res.tile([HID, 1], dt.bfloat16)
            invc = res.tile([P, 1], dt.float32)
            iota_b = res.tile([P, P], dt.bfloat16)
            ident = res.tile([P, P], dt.bfloat16)
            b1r = res.tile([1, F1], dt.bfloat16)
            b2r = res.tile([1, F2], dt.bfloat16)
            b3r = res.tile([1, F3], dt.bfloat16)
            ones1 = res.tile([1, P], dt.bfloat16)
            nc.vector.memset(ones1[:], 1.0)
            for sb, dr in ((eidx, eidx_d), (enrm, enrm_d), (edl, edl_d),
                           (gloc, gloc_d), (w1a, w1a_d), (w2a, w2a_d),
                           (w3a, w3a_d), (fb1c, fb1_d), (fw2, fw2_d),
                           (invc, invc_d), (iota_b, iota_d), (ident, ident_d)):
                nc.sync.dma_start(out=sb[:], in_=dr[:])
            nc.sync.dma_start(out=b1r[:], in_=w1a_d[F0:F0 + 1, :])
            nc.sync.dma_start(out=b2r[:], in_=w2a_d[F1:F1 + 1, :])
            nc.sync.dma_start(out=b3r[:], in_=w3a_d[F2:F2 + 1, :])
            nc.sync.dma_start(out=fw1a[:], in_=fw1_d[0:F3 // 2, :])
            nc.sync.dma_start(out=fw1b[:], in_=fw1_d[F3 // 2:, :])

            # ---- DRAM tables / buffers (all padded tables have 256B rows)
            xt = dram.tile([NPAD, P], dt.bfloat16)       # padded x
            h1s = dram.tile([NW, P, F1], dt.bfloat16)    # compact shard out L1
            h2s = dram.tile([NW, P, F2], dt.bfloat16)
            h1c = dram.tile([NPAD, F1], dt.bfloat16)     # compact gathered
            h2c = dram.tile([NPAD, F2], dt.bfloat16)
            h1t = dram.tile([NPAD, P], dt.bfloat16)      # padded tables
            h2t = dram.tile([NPAD, P], dt.bfloat16)
            pool_pt = dram.tile([P, F3], dt.float32)
            pool_rd = dram.tile([P, F3], dt.float32)

            # expand compact x -> padded (per segment, overlaps other loads)
            for s in range(NSEG):
                a, b = REG_BASE[s], REG_BASE[s] + REG_SIZE[s]
                nc.sync.dma_start(out=xt[a:b, :F0], in_=xtc_d[a:b, :])

            pool_acc = pool_ps.tile([P, F3], dt.float32)

            # cols per dma_gather: with single_packet=False the ucode handles
            # ~8k idxs/inst; >=17920 crashes. 40 cols = 5120 idxs is safe.
            GMAX = 20

            def gathers(tbl, F_in, p):
                """Issue the per-segment dma_gathers for piece p (chunked to
                <=GMAX*128 idxs per instruction); returns the msgs tile."""
                msgs = msgsp.tile([P, st.cmax, P], dt.bfloat16, tag="msgs",
                                  name="msgs")
                s0 = st.piece_col0[p] * P
                for s in range(NSEG):
                    loff, ncols = st.gath[p][s]
                    for c0 in range(0, ncols, GMAX):
                        nc_ = min(GMAX, ncols - c0)
                        ni = nc_ * P
                        slot0 = s0 + (loff + c0) * P
                        nc.gpsimd.dma_gather(
                            out_ap=msgs[:, loff + c0:loff + c0 + nc_, :],
                            in_ap=tbl[REG_BASE[s]:REG_BASE[s] + REG_SIZE[s], :],
                            idxs_ap=eidx[:, slot0 // 16:(slot0 + ni) // 16],
                            num_idxs=ni, num_idxs_reg=ni, elem_size=P,
                            single_packet=False,
                            queue_num=(s + c0 // GMAX) % 4)
                return msgs

            def compute_piece(msgs, F_in, F_out, waug, brow, shard_out, p,
                              last):
                c0 = st.piece_col0[p]
                cn = st.piece_ncol[p]
                mw = msgs[:, :cn, :F_in]
                nc.vector.tensor_tensor(
                    out=mw, in0=mw,
                    in1=enrm[:, c0:c0 + cn, None].broadcast_to([P, cn, F_in]),
                    op=OP.mult)
                for wi in range(CW):
                    w = p * CW + wi
                    runs = st.wruns[p * CW + wi]
                    tw = sum(rn[1] for rn in runs)
                    S = sp.tile([P, TMAX, P], dt.bfloat16, tag="S", name="S")
                    joff = 0
                    for (loff, rn, gcol) in runs:
                        nc.vector.tensor_tensor(
                            out=S[:, joff:joff + rn, :],
                            in0=edl[:, gcol:gcol + rn, None].broadcast_to(
                                [P, rn, P]),
                            in1=iota_b[:, None, :].broadcast_to([P, rn, P]),
                            op=OP.is_equal)
                        joff += rn
                    pa = pa_ps.tile([FMAX, P], dt.float32, tag="pa",
                                    name="pa")[:F_in]
                    k = 0
                    for (loff, rn, gcol) in runs:
                        for j in range(rn):
                            nc.tensor.matmul(
                                out=pa[:], lhsT=msgs[:, loff + j, :F_in],
                                rhs=S[:, k, :], start=(k == 0),
                                stop=(k == tw - 1))
                            k += 1
                    aggT = work.tile([FMAX, P], dt.bfloat16, tag="aggT",
                                     name="aggT")[:F_in]
                    nc.scalar.copy(out=aggT[:], in_=pa[:])
                    p2 = p2_ps.tile([P, F3], dt.float32, tag="p2",
                                    name="p2")[:, :F_out]
                    nc.tensor.matmul(out=p2[:], lhsT=aggT[:], rhs=waug[:F_in, :],
                                     start=True, stop=False)
                    nc.tensor.matmul(out=p2[:], lhsT=ones1[:], rhs=brow[:],
                                     start=False, stop=True)
                    h = hwp.tile([P, F3], dt.bfloat16, tag="h", name="h")[:, :F_out]
                    nc.scalar.activation(h[:], p2[:], AF.Relu)
                    if not last:
                        nc.sync.dma_start(out=shard_out[w, :, :], in_=h[:])
                    else:
                        Sg = sp.tile([P, P], dt.bfloat16, tag="Sg", name="Sg")
                        nc.vector.tensor_tensor(
                            out=Sg[:],
                            in0=gloc[:, w:w + 1].broadcast_to([P, P]),
                            in1=iota_b[:], op=OP.is_equal)
                        nc.tensor.matmul(out=pool_acc[:], lhsT=Sg[:], rhs=h[:],
                                         start=(w == 0), stop=(w == NW - 1))

            def allgather_seg(shard, ctab, s):
                a, b = REG_BASE[s], REG_BASE[s] + REG_SIZE[s]
                nc.gpsimd.collective_compute(
                    "AllGather", mybir.AluOpType.bypass, replica_groups=rg,
                    ins=[shard[SEG_WSTART[s]:SEG_WSTART[s] + SEG_W[s], :, :].opt()],
                    outs=[ctab[a:b, :].opt()])

            def expand(ctab, ptab, F_out):
                for s in range(NSEG):
                    a, b = REG_BASE[s], REG_BASE[s] + REG_SIZE[s]
                    nc.sync.dma_start(out=ptab[a:b, :F_out], in_=ctab[a:b, :])

            def layer(tbl, F_in, F_out, waug, brow, shard_out, ag):
                # ag(s): AllGather of THIS layer's output segment s; issued on
                # the Pool queue two pieces after segment s's last piece so
                # the trigger's wait on compute h-writes doesn't stall the
                # gather stream. Segment 3's fires after the loop.
                last = shard_out is None
                for p in range(NP):
                    msgs = gathers(tbl, F_in, p)
                    if ag is not None:
                        for s in range(3):
                            if p == 4 * s + 5:
                                ag(s)
                    compute_piece(msgs, F_in, F_out, waug, brow, shard_out,
                                  p, last)
                if ag is not None:
                    ag(3)

            dbg_layers = int(os.environ.get("GCN_DEBUG_LAYERS", "3"))
            n_repeat = int(os.environ.get("GCN_REPEAT", "1"))
            for _rep in range(n_repeat):
                if dbg_layers >= 1:
                    layer(xt, F0, F1, w1a, b1r, h1s,
                          (lambda s: allgather_seg(h1s, h1c, s))
                          if dbg_layers >= 2 else None)
                if dbg_layers >= 2:
                    expand(h1c, h1t, F1)
                    layer(h1t, F1, F2, w2a, b2r, h2s,
                          (lambda s: allgather_seg(h2s, h2c, s))
                          if dbg_layers >= 3 else None)
                if dbg_layers >= 3:
                    expand(h2c, h2t, F2)
                    layer(h2t, F2, F3, w3a, b3r, None, None)
                if dbg_layers < 3:
                    # stub tail: emit outputs without pool/head
                    dsrc = h1s if dbg_layers >= 1 else None
                    pz = work.tile([P, F3], dt.float32, tag="poolr")
                    if dsrc is not None:
                        nc.vector.memset(pz[:], 0.0)
                        hdbg = work.tile([P, F1], dt.bfloat16, tag="hdbg")
                        nc.sync.dma_start(out=hdbg[:], in_=dsrc[0, :, :])
                        nc.vector.tensor_copy(out=pz[:, :F1], in_=hdbg[:])
                    else:
                        nc.vector.memset(pz[:], 0.0)
                    nc.sync.dma_start(out=pdbg_d[:], in_=pz[:])
                    o1 = work.tile([1, P], dt.float32, tag="outs")
                    nc.vector.memset(o1[:], 1.0)
                    nc.sync.dma_start(out=out_d[:], in_=o1[:])

                if dbg_layers >= 3:
                    # ---- pooling partial -> AllReduce -> mean
                    psb = work.tile([P, F3], dt.float32, tag="psb")
                    nc.scalar.copy(out=psb[:], in_=pool_acc[:])
                    nc.sync.dma_start(out=pool_pt[:], in_=psb[:])
                    nc.gpsimd.collective_compute(
                        "AllReduce", mybir.AluOpType.add, replica_groups=rg,
                        ins=[pool_pt.opt()], outs=[pool_rd.opt()])
                    poolr = work.tile([P, F3], dt.float32, tag="poolr")
                    nc.sync.dma_start(out=poolr[:], in_=pool_rd[:])
                    nc.sync.dma_start(out=pdbg_d[:], in_=poolr[:])
                    pooled = work.tile([P, F3], dt.bfloat16, tag="pooled")
                    nc.scalar.activation(pooled[:], poolr[:], AF.Copy, scale=invc[:])

                    # ---- head: z1 = relu(pooled @ fW1 + fb1); z2 = z1 @ fW2 + fb2
                    ptA_ps = head_ps.tile([F3 // 2, P], dt.bfloat16, tag="pt")
                    nc.tensor.transpose(out=ptA_ps[:], in_=pooled[:, :F3 // 2],
                                        identity=ident[:])
                    ptA = work.tile([F3 // 2, P], dt.bfloat16, tag="ptA")
                    nc.scalar.copy(out=ptA[:], in_=ptA_ps[:])
                    ptB_ps = head_ps.tile([F3 // 2, P], dt.bfloat16, tag="pt")
                    nc.tensor.transpose(out=ptB_ps[:], in_=pooled[:, F3 // 2:],
                                        identity=ident[:])
                    ptB = work.tile([F3 // 2, P], dt.bfloat16, tag="ptB")
                    nc.scalar.copy(out=ptB[:], in_=ptB_ps[:])

                    z1_ps = head_ps.tile([HID, P], dt.float32, tag="z1")
                    nc.tensor.matmul(out=z1_ps[:], lhsT=fw1a[:], rhs=ptA[:],
                                     start=True, stop=False)
                    nc.tensor.matmul(out=z1_ps[:], lhsT=fw1b[:], rhs=ptB[:],
                                     start=False, stop=True)
                    z1 = work.tile([HID, P], dt.bfloat16, tag="z1s")
                    nc.scalar.activation(z1[:], z1_ps[:], AF.Relu, bias=fb1c[:])

                    z2_ps = head_ps.tile([1, P], dt.float32, tag="z2")
                    nc.tensor.matmul(out=z2_ps[:], lhsT=fw2[:], rhs=z1[:],
                                     start=True, stop=True)
                    z2 = work.tile([1, P], dt.float32, tag="z2s")
                    nc.scalar.activation(z2[:], z2_ps[:], AF.Copy, bias=float(fb2))
                    # softmax over a width-1 axis == 1.0 for finite logits
                    outs = work.tile([1, P], dt.float32, tag="outs")
                    nc.vector.tensor_tensor(out=outs[:], in0=z2[:], in1=z2[:],
                                            op=OP.is_equal)
                    nc.sync.dma_start(out=out_d[:], in_=outs[:])

    nc.compile()
    return nc


# ---------------------------------------------------------------- run

_CACHE = {}


def _get_nc(st, fb2):
    import os as _os
    key = (st.key(), fb2, _os.environ.get('GCN_DEBUG_LAYERS', '3'))
    if key not in _CACHE:
        _CACHE[key] = build_bass(st, fb2)
    return _CACHE[key]


def make_in_maps(inputs):
    st, cores, wts, fb2 = build_host_data(inputs)
    in_maps = [dict(**cores[c], **wts) for c in range(NCORES)]
    return st, in_maps, fb2


LAST_RESULTS = None


def kernel(**inputs):
    global LAST_RESULTS
    st, in_maps, fb2 = make_in_maps(inputs)
    nc = _get_nc(st, fb2)
    from concourse.bass_utils import run_bass_kernel_spmd
    res = run_bass_kernel_spmd(nc, in_maps, core_ids=list(range(NCORES)))
    LAST_RESULTS = res
    out = np.asarray(res.results[0]["out"]).reshape(P)[:G]
    return out.reshape(G, 1).astype(np.float32)



# revision 9
# speedup vs baseline: 37.2091x; 37.2091x over previous
"""GCN (3-layer GCNConv + mean-pool + MLP head) Trainium2 Bass kernel, 8 NeuronCores.

Strategy (graph/data parallel, per sharding hint):
  - Destination nodes are partitioned into 8 contiguous blocks (one per core);
    each core owns SHARD=12544 destinations = 98 windows of 128.
  - Node features live in DRAM tables with 256B rows ([NPAD, 128] bf16, first
    F_in columns real) so the custom dma_gather instruction (InstDMAGatherAnt)
    can fetch thousands of source rows per instruction. Gathers are chunked to
    <=5120 idxs and spread round-robin over the 4 SWDGE queues so the Q7
    descriptor generation (2 cores per queue) runs 4-wide.
  - Symmetric GCN norm factoring: norm(e) = dis[src]*dis[dst] with
    dis = deg^-1/2. The layer tables store h~ = dis*h (the src factor); the
    dst factor is applied per window: bias matmul uses lhsT = sqrt(deg_d)
    (so relu(dis*(agg@W) + dis*sdeg*b) = relu(...+b)), and the storage/pool
    activation uses scale=dis (relu(dis*x) = dis*relu(x) since dis>0). This
    removes the per-edge norm multiply and the edge-norm table entirely.
  - The node id space is remapped segment-major into 4 regions of <=28672
    rows so gather indices fit int16; edges are bucketed by (piece of 7 dst
    windows, source segment, dst window), each bucket padded to full
    128-edge columns (counts maxed over cores so the SPMD program is
    identical on every core). Bucket entries are sorted by source row for
    better HBM locality during the gather drain. Self-loops are regular
    edges.
  - Per window a one-hot S matmul chain accumulates messages in PSUM
    (aggregate-then-transform).
  - Layer outputs are written as FULL 256B rows into a compact shard buffer
    and AllGathered per segment directly into the next layer's padded gather
    table (no separate expand step; the AllGather moves 256B rows).
  - Layer-3 output is mean-pooled per graph via one-hot matmuls into a PSUM
    accumulator, AllReduced, and the tiny FC head runs replicated.
"""

import os
import sys
from dataclasses import dataclass

import numpy as np
import ml_dtypes

for _p in ("/opt/trn_rl_repo", "/root/.axon_site/_ro/trn_rl_repo"):
    if os.path.isdir(_p) and _p not in sys.path:
        sys.path.insert(0, _p)

bf16 = ml_dtypes.bfloat16
P = 128
N = 100000
G = 128
F = (40, 40, 80, 160)
HID = 128
NCORES = 8
SHARD = 12544
NW = 98                       # windows per core
CW = 7                        # windows per piece
NP = NW // CW                 # 14 pieces
SEG_W = (28, 28, 28, 14)      # windows per segment
SEG_NODES = tuple(w * P for w in SEG_W)            # 3584,3584,3584,1792
SEG_WSTART = (0, 28, 56, 84)
SEG_START = tuple(w * P for w in SEG_WSTART)       # node offset within shard
REG_SIZE = tuple(NCORES * n for n in SEG_NODES)    # 28672*3, 14336
REG_BASE = (0, 28672, 57344, 86016)
NPAD = NCORES * SHARD         # 100352
NSEG = 4


# ---------------------------------------------------------------- host prep

def _remap_rows():
    """node id -> segment-major global table row."""
    v = np.arange(NPAD, dtype=np.int64)
    c, r = v // SHARD, v % SHARD
    s = np.minimum(r // SEG_NODES[0], 3)
    row = (np.asarray(REG_BASE)[s] + c * np.asarray(SEG_NODES)[s]
           + (r - np.asarray(SEG_START)[s]))
    return row


@dataclass
class Structure:
    ncol: np.ndarray      # [NW, NSEG] columns per (window, seg) bucket
    totcol: int
    totslot: int
    cmax: int             # max columns in a piece
    piece_col0: list      # per piece: first global column
    piece_ncol: list      # per piece: total columns
    gath: list            # per piece: list over seg of (local col off, ncols)
    wruns: list           # per (piece, wi): list of (local col off, ncols, gcol0)

    def key(self):
        return (self.ncol.tobytes(), self.totcol, self.cmax)


def build_structure(ncol):
    """Static (SPMD-uniform) grid layout from the per-(window,seg) column
    counts. Column order: piece-major, then seg, then window."""
    piece_col0, piece_ncol, gath, wruns = [], [], [], []
    col = 0
    for p in range(NP):
        piece_col0.append(col)
        pg = []
        local = 0
        runs = {wi: [] for wi in range(CW)}
        for s in range(NSEG):
            nc_s = 0
            for wi in range(CW):
                w = p * CW + wi
                n = int(ncol[w, s])
                if n:
                    runs[wi].append((local + nc_s, n, col + local + nc_s))
                nc_s += n
            pg.append((local, nc_s))
            local += nc_s
        piece_ncol.append(local)
        gath.append(pg)
        for wi in range(CW):
            wruns.append(runs[wi])
        col += local
    totcol = col
    return Structure(ncol=ncol, totcol=totcol, totslot=totcol * P,
                     cmax=max(piece_ncol), piece_col0=piece_col0,
                     piece_ncol=piece_ncol, gath=gath, wruns=wruns)


def build_host_data(inp):
    src = np.asarray(inp["edge_index"][0]).astype(np.int64).ravel()
    dst = np.asarray(inp["edge_index"][1]).astype(np.int64).ravel()
    batch = np.asarray(inp["batch"]).astype(np.int64).ravel()
    deg = (np.bincount(dst, minlength=N) + 1).astype(np.float64)
    dis = 1.0 / np.sqrt(deg)
    # self-loops folded in as regular edges; norm factoring makes them free
    loop = np.arange(N, dtype=np.int64)
    srcA = np.concatenate([src, loop])
    dstA = np.concatenate([dst, loop])

    remap = _remap_rows()
    srow = remap[srcA]
    seg = np.minimum(srow // REG_SIZE[0], 3)
    lidx = srow - np.asarray(REG_BASE)[seg]            # int16-safe (<28672)

    core = dstA // SHARD
    r = dstA % SHARD
    w = r // P
    dloc = r % P
    p = w // CW

    # per-core per-(w,seg) counts -> uniform column counts (max over cores)
    kid_full = ((core * NW + w) * NSEG + seg)
    cnt = np.bincount(kid_full, minlength=NCORES * NW * NSEG).reshape(
        NCORES, NW, NSEG)
    ncol = np.ceil(cnt.max(axis=0) / P).astype(np.int64)   # [NW, NSEG]
    ncol = np.maximum(ncol, 1)
    st = build_structure(ncol)

    # bucket start slots in (piece, seg, window) order
    bstart = np.zeros((NW, NSEG), np.int64)
    colc = 0
    for pp in range(NP):
        for s in range(NSEG):
            for wi in range(CW):
                ww = pp * CW + wi
                bstart[ww, s] = colc * P
                colc += int(ncol[ww, s])
    assert colc == st.totcol

    cores = []
    for c in range(NCORES):
        m = core == c
        sl, dl, ww, ss = lidx[m], dloc[m], w[m], seg[m]
        # sort by (piece, seg, window) = layout order; src row as tiebreak
        # so each bucket's gather reads ascend through HBM
        okey = (ww // CW) * (NSEG * NW) + ss * NW + ww
        order = np.lexsort((sl, okey))
        sl, dl, ww, ss = (a[order] for a in (sl, dl, ww, ss))
        bid = ww * NSEG + ss
        cnts = np.bincount(bid, minlength=NW * NSEG)
        # j within bucket
        sk = np.argsort(bid, kind="stable")
        inv = np.empty_like(sk)
        inv[sk] = np.arange(len(sk))
        csum = np.concatenate([[0], np.cumsum(cnts)])
        j = inv - csum[bid]
        slot = bstart[ww, ss] + j
        eidx = np.zeros(st.totslot, np.int16)
        edl = np.full(st.totslot, -1.0, np.float32)
        eidx[slot] = sl.astype(np.int16)
        edl[slot] = dl.astype(np.float32)
        # idx wrap: slot i -> [i%16, i//16], replicated over 8 groups of 16
        idx16 = eidx.reshape(-1, 16).T.copy()          # [16, totslot/16]
        idx_full = np.tile(idx16, (8, 1))
        # grid wrap: slot i -> [i%128, i//128]
        edl_g = np.ascontiguousarray(
            edl.reshape(-1, P).T).astype(bf16)
        nid = np.arange(SHARD) + c * SHARD
        gl = np.where(nid < N, batch[np.minimum(nid, N - 1)], -1.0)
        gloc = np.ascontiguousarray(
            gl.reshape(NW, P).T).astype(bf16)          # [128, NW]
        disn = np.where(nid < N, dis[np.minimum(nid, N - 1)], 1.0)
        disv = np.ascontiguousarray(
            disn.reshape(NW, P).T).astype(np.float32)  # [128, NW], dis
        disq = np.ascontiguousarray(
            (disn * disn).reshape(NW, P).T).astype(np.float32)  # 1/deg
        sdeg = (1.0 / disn).astype(bf16).reshape(1, SHARD)
        cores.append(dict(eidx=idx_full, edl=edl_g, gloc=gloc,
                          disv=disv, disq=disq, sdeg=sdeg))

    # padded x table in remapped row order, pre-scaled by dis[src]
    xt = np.zeros((NPAD, P), bf16)
    xt[remap[:N], :F[0]] = (np.asarray(inp["x"], np.float64)
                            * dis[:, None]).astype(bf16)

    cnt_g = np.bincount(batch, minlength=G).astype(np.float32)
    invc = np.zeros((P, 1), np.float32)
    invc[:G, 0] = 1.0 / np.maximum(cnt_g, 1.0)

    iota = np.tile(np.arange(P, dtype=np.float32), (P, 1)).astype(bf16)
    ident = np.eye(P, dtype=np.float32).astype(bf16)

    def a2(x, dt):
        return np.ascontiguousarray(np.asarray(x), dtype=dt)

    wts = dict(
        w1a=np.concatenate([a2(inp["W1"], bf16), a2(inp["b1"], bf16)[None]], 0),
        w2a=np.concatenate([a2(inp["W2"], bf16), a2(inp["b2"], bf16)[None]], 0),
        w3a=np.concatenate([a2(inp["W3"], bf16), a2(inp["b3"], bf16)[None]], 0),
        fw1=a2(inp["fW1"], bf16),
        fb1c=a2(inp["fb1"], np.float32).reshape(-1, 1),
        fw2=a2(inp["fW2"], bf16),
        invc=invc,
        iota=iota,
        ident=ident,
        xt=xt,
    )
    fb2 = float(np.asarray(inp["fb2"]).ravel()[0])
    return st, cores, wts, fb2


# ---------------------------------------------------------------- bass build

def build_bass(st, fb2):
    import concourse.bacc as bacc
    import concourse.bass as bass
    import concourse.mybir as mybir
    import concourse.tile as tile

    dt = mybir.dt
    AF = mybir.ActivationFunctionType
    OP = mybir.AluOpType
    F0, F1, F2, F3 = F
    FMAX = max(F0, F1, F2)
    TMAX = int(st.ncol.sum(axis=1).max())    # max columns per window

    nc = bacc.Bacc("TRN2", target_bir_lowering=False, debug=False,
                   enable_asserts=False, num_devices=NCORES,
                   num_swdge_queues=4)

    # ---- I/O
    xt_d = nc.dram_tensor("xt", [NPAD, P], dt.bfloat16, kind="ExternalInput")
    eidx_d = nc.dram_tensor("eidx", [P, st.totslot // 16], dt.int16,
                            kind="ExternalInput")
    edl_d = nc.dram_tensor("edl", [P, st.totcol], dt.bfloat16,
                           kind="ExternalInput")
    gloc_d = nc.dram_tensor("gloc", [P, NW], dt.bfloat16, kind="ExternalInput")
    disv_d = nc.dram_tensor("disv", [P, NW], dt.float32, kind="ExternalInput")
    disq_d = nc.dram_tensor("disq", [P, NW], dt.float32, kind="ExternalInput")
    sdeg_d = nc.dram_tensor("sdeg", [1, SHARD], dt.bfloat16,
                            kind="ExternalInput")
    w1a_d = nc.dram_tensor("w1a", [F0 + 1, F1], dt.bfloat16, kind="ExternalInput")
    w2a_d = nc.dram_tensor("w2a", [F1 + 1, F2], dt.bfloat16, kind="ExternalInput")
    w3a_d = nc.dram_tensor("w3a", [F2 + 1, F3], dt.bfloat16, kind="ExternalInput")
    fw1_d = nc.dram_tensor("fw1", [F3, HID], dt.bfloat16, kind="ExternalInput")
    fb1_d = nc.dram_tensor("fb1c", [HID, 1], dt.float32, kind="ExternalInput")
    fw2_d = nc.dram_tensor("fw2", [HID, 1], dt.bfloat16, kind="ExternalInput")
    invc_d = nc.dram_tensor("invc", [P, 1], dt.float32, kind="ExternalInput")
    iota_d = nc.dram_tensor("iota", [P, P], dt.bfloat16, kind="ExternalInput")
    ident_d = nc.dram_tensor("ident", [P, P], dt.bfloat16, kind="ExternalInput")
    out_d = nc.dram_tensor("out", [1, P], dt.float32, kind="ExternalOutput")
    pdbg_d = nc.dram_tensor("pooled_dbg", [P, F3], dt.float32,
                            kind="ExternalOutput")

    rg = [list(range(NCORES))]

    with tile.TileContext(nc) as tc:
        with (
            tc.tile_pool(name="res", bufs=1) as res,
            tc.tile_pool(name="msgs", bufs=3) as msgsp,
            tc.tile_pool(name="sp", bufs=3) as sp,
            tc.tile_pool(name="work", bufs=3) as work,
            tc.tile_pool(name="hw", bufs=3) as hwp,
            tc.tile_pool(name="pa_ps", bufs=2, space="PSUM") as pa_ps,
            tc.tile_pool(name="p2_ps", bufs=2, space="PSUM") as p2_ps,
            tc.tile_pool(name="pool_ps", bufs=1, space="PSUM") as pool_ps,
            tc.tile_pool(name="head_ps", bufs=1, space="PSUM") as head_ps,
            tc.tile_pool(name="dram", bufs=1, space="DRAM") as dram,
        ):
            # ---- persistent SBUF state
            eidx = res.tile([P, st.totslot // 16], dt.int16)
            edl = res.tile([P, st.totcol], dt.bfloat16)
            gloc = res.tile([P, NW], dt.bfloat16)
            disv = res.tile([P, NW], dt.float32)
            disq = res.tile([P, NW], dt.float32)
            sdeg = res.tile([1, SHARD], dt.bfloat16)
            w1a = res.tile([F0 + 1, F1], dt.bfloat16)
            w2a = res.tile([F1 + 1, F2], dt.bfloat16)
            w3a = res.tile([F2 + 1, F3], dt.bfloat16)
            fw1a = res.tile([F3 // 2, HID], dt.bfloat16)
            fw1b = res.tile([F3 // 2, HID], dt.bfloat16)
            fb1c = res.tile([HID, 1], dt.float32)
            fw2 = res.tile([HID, 1], dt.bfloat16)
            invc = res.tile([P, 1], dt.float32)
            iota_b = res.tile([P, P], dt.bfloat16)
            ident = res.tile([P, P], dt.bfloat16)
            b1r = res.tile([1, F1], dt.bfloat16)
            b2r = res.tile([1, F2], dt.bfloat16)
            b3r = res.tile([1, F3], dt.bfloat16)
            for sb, dr in ((eidx, eidx_d), (edl, edl_d),
                           (gloc, gloc_d), (disv, disv_d), (disq, disq_d),
                           (sdeg, sdeg_d),
                           (w1a, w1a_d), (w2a, w2a_d),
                           (w3a, w3a_d), (fb1c, fb1_d), (fw2, fw2_d),
                           (invc, invc_d), (iota_b, iota_d), (ident, ident_d)):
                nc.sync.dma_start(out=sb[:], in_=dr[:])
            nc.sync.dma_start(out=b1r[:], in_=w1a_d[F0:F0 + 1, :])
            nc.sync.dma_start(out=b2r[:], in_=w2a_d[F1:F1 + 1, :])
            nc.sync.dma_start(out=b3r[:], in_=w3a_d[F2:F2 + 1, :])
            nc.sync.dma_start(out=fw1a[:], in_=fw1_d[0:F3 // 2, :])
            nc.sync.dma_start(out=fw1b[:], in_=fw1_d[F3 // 2:, :])

            # ---- DRAM tables / buffers (gather tables have 256B rows)
            h1s = dram.tile([NW, P, P], dt.bfloat16)     # shard out, 256B rows
            h2s = dram.tile([NW, P, P], dt.bfloat16)
            h1t = dram.tile([NPAD, P], dt.bfloat16)
            h2t = dram.tile([NPAD, P], dt.bfloat16)
            pool_pt = dram.tile([P, F3], dt.float32)
            pool_rd = dram.tile([P, F3], dt.float32, addr_space="Shared")

            pool_acc = pool_ps.tile([P, F3], dt.float32)

            # cols per dma_gather: with single_packet=False the ucode handles
            # ~8k idxs/inst; >=17920 crashes. 40 cols = 5120 idxs is safe.
            GMAX = 40
            qctr = [0]

            def gathers(tbl, F_in, p):
                """Issue the per-segment dma_gathers for piece p (chunked to
                <=GMAX*128 idxs per instruction); returns the msgs tile."""
                msgs = msgsp.tile([P, st.cmax, P], dt.bfloat16, tag="msgs",
                                  name="msgs")
                s0 = st.piece_col0[p] * P
                for s in range(NSEG):
                    loff, ncols = st.gath[p][s]
                    for c0 in range(0, ncols, GMAX):
                        nc_ = min(GMAX, ncols - c0)
                        ni = nc_ * P
                        slot0 = s0 + (loff + c0) * P
                        nc.gpsimd.dma_gather(
                            out_ap=msgs[:, loff + c0:loff + c0 + nc_, :],
                            in_ap=tbl[REG_BASE[s]:REG_BASE[s] + REG_SIZE[s], :],
                            idxs_ap=eidx[:, slot0 // 16:(slot0 + ni) // 16],
                            num_idxs=ni, num_idxs_reg=ni, elem_size=P,
                            single_packet=False,
                            queue_num=qctr[0] % 4)
                        qctr[0] += 1
                return msgs

            def compute_piece(msgs, F_in, F_out, waug, brow, shard_out, p,
                              last):
                for wi in range(CW):
                    w = p * CW + wi
                    runs = st.wruns[p * CW + wi]
                    tw = sum(rn[1] for rn in runs)
                    S = sp.tile([P, TMAX, P], dt.bfloat16, tag="S", name="S")
                    joff = 0
                    for (loff, rn, gcol) in runs:
                        nc.vector.tensor_tensor(
                            out=S[:, joff:joff + rn, :],
                            in0=edl[:, gcol:gcol + rn, None].broadcast_to(
                                [P, rn, P]),
                            in1=iota_b[:, None, :].broadcast_to([P, rn, P]),
                            op=OP.is_equal)
                        joff += rn
                    pa = pa_ps.tile([FMAX, P], dt.float32, tag="pa",
                                    name="pa")[:F_in]
                    k = 0
                    for (loff, rn, gcol) in runs:
                        for j in range(rn):
                            nc.tensor.matmul(
                                out=pa[:], lhsT=msgs[:, loff + j, :F_in],
                                rhs=S[:, k, :], start=(k == 0),
                                stop=(k == tw - 1))
                            k += 1
                    aggT = work.tile([FMAX, P], dt.bfloat16, tag="aggT",
                                     name="aggT")[:F_in]
                    nc.scalar.copy(out=aggT[:], in_=pa[:])
                    p2 = p2_ps.tile([P, F3], dt.float32, tag="p2",
                                    name="p2")[:, :F_out]
                    nc.tensor.matmul(out=p2[:], lhsT=aggT[:], rhs=waug[:F_in, :],
                                     start=True, stop=False)
                    nc.tensor.matmul(out=p2[:],
                                     lhsT=sdeg[:, w * P:(w + 1) * P],
                                     rhs=brow[:], start=False, stop=True)
                    if not last:
                        # store dis*relu(agg@W+b) = relu(dis^2 * p2)
                        h = hwp.tile([P, P], dt.bfloat16, tag="h", name="h")
                        nc.scalar.activation(h[:, :F_out], p2[:], AF.Relu,
                                             scale=disq[:, w:w + 1])
                        nc.sync.dma_start(out=shard_out[w, :, :], in_=h[:])
                    else:
                        h = hwp.tile([P, F3], dt.bfloat16, tag="h", name="h")
                        nc.scalar.activation(h[:], p2[:], AF.Relu,
                                             scale=disv[:, w:w + 1])
                        Sg = sp.tile([P, P], dt.bfloat16, tag="Sg", name="Sg")
                        nc.vector.tensor_tensor(
                            out=Sg[:],
                            in0=gloc[:, w:w + 1].broadcast_to([P, P]),
                            in1=iota_b[:], op=OP.is_equal)
                        nc.tensor.matmul(out=pool_acc[:], lhsT=Sg[:], rhs=h[:],
                                         start=(w == 0), stop=(w == NW - 1))

            def allgather_seg(shard, ptab, s):
                a, b = REG_BASE[s], REG_BASE[s] + REG_SIZE[s]
                nc.gpsimd.collective_compute(
                    "AllGather", mybir.AluOpType.bypass, replica_groups=rg,
                    ins=[shard[SEG_WSTART[s]:SEG_WSTART[s] + SEG_W[s], :, :].opt()],
                    outs=[ptab[a:b, :].opt()])

            def layer(tbl, F_in, F_out, waug, brow, shard_out, ag):
                # ag(s): AllGather of THIS layer's output segment s; issued on
                # the Pool queue two pieces after segment s's last piece so
                # the trigger's wait on compute h-writes doesn't stall the
                # gather stream. Segment 3's fires after the loop.
                last = shard_out is None
                for p in range(NP):
                    msgs = gathers(tbl, F_in, p)
                    if ag is not None:
                        for s in range(3):
                            if p == 4 * s + 5:
                                ag(s)
                    compute_piece(msgs, F_in, F_out, waug, brow, shard_out,
                                  p, last)
                if ag is not None:
                    ag(3)

            dbg_layers = int(os.environ.get("GCN_DEBUG_LAYERS", "3"))
            n_repeat = int(os.environ.get("GCN_REPEAT", "1"))
            for _rep in range(n_repeat):
                if dbg_layers >= 1:
                    layer(xt_d, F0, F1, w1a, b1r, h1s,
                          (lambda s: allgather_seg(h1s, h1t, s))
                          if dbg_layers >= 2 else None)
                if dbg_layers >= 2:
                    layer(h1t, F1, F2, w2a, b2r, h2s,
                          (lambda s: allgather_seg(h2s, h2t, s))
                          if dbg_layers >= 3 else None)
                if dbg_layers >= 3:
                    layer(h2t, F2, F3, w3a, b3r, None, None)
                if dbg_layers < 3:
                    # stub tail: emit outputs without pool/head
                    dsrc = h1s if dbg_layers >= 1 else None
                    pz = work.tile([P, F3], dt.float32, tag="poolr")
                    if dsrc is not None:
                        nc.vector.memset(pz[:], 0.0)
                        hdbg = work.tile([P, F1], dt.bfloat16, tag="hdbg")
                        nc.sync.dma_start(out=hdbg[:], in_=dsrc[0, :, :F1])
                        nc.vector.tensor_copy(out=pz[:, :F1], in_=hdbg[:])
                    else:
                        nc.vector.memset(pz[:], 0.0)
                    nc.sync.dma_start(out=pdbg_d[:], in_=pz[:])
                    o1 = work.tile([1, P], dt.float32, tag="outs")
                    nc.vector.memset(o1[:], 1.0)
                    nc.sync.dma_start(out=out_d[:], in_=o1[:])

                if dbg_layers >= 3:
                    # ---- pooling partial -> AllReduce -> mean
                    psb = work.tile([P, F3], dt.float32, tag="psb")
                    nc.scalar.copy(out=psb[:], in_=pool_acc[:])
                    nc.sync.dma_start(out=pool_pt[:], in_=psb[:])
                    nc.gpsimd.collective_compute(
                        "AllReduce", mybir.AluOpType.add, replica_groups=rg,
                        ins=[pool_pt.opt()], outs=[pool_rd.opt()])
                    poolr = work.tile([P, F3], dt.float32, tag="poolr")
                    nc.sync.dma_start(out=poolr[:], in_=pool_rd[:])
                    nc.sync.dma_start(out=pdbg_d[:], in_=poolr[:])
                    pooled = work.tile([P, F3], dt.bfloat16, tag="pooled")
                    nc.scalar.activation(pooled[:], poolr[:], AF.Copy, scale=invc[:])

                    # ---- head: z1 = relu(pooled @ fW1 + fb1); z2 = z1 @ fW2 + fb2
                    ptA_ps = head_ps.tile([F3 // 2, P], dt.bfloat16, tag="pt")
                    nc.tensor.transpose(out=ptA_ps[:], in_=pooled[:, :F3 // 2],
                                        identity=ident[:])
                    ptA = work.tile([F3 // 2, P], dt.bfloat16, tag="ptA")
                    nc.scalar.copy(out=ptA[:], in_=ptA_ps[:])
                    ptB_ps = head_ps.tile([F3 // 2, P], dt.bfloat16, tag="pt")
                    nc.tensor.transpose(out=ptB_ps[:], in_=pooled[:, F3 // 2:],
                                        identity=ident[:])
                    ptB = work.tile([F3 // 2, P], dt.bfloat16, tag="ptB")
                    nc.scalar.copy(out=ptB[:], in_=ptB_ps[:])

                    z1_ps = head_ps.tile([HID, P], dt.float32, tag="z1")
                    nc.tensor.matmul(out=z1_ps[:], lhsT=fw1a[:], rhs=ptA[:],
                                     start=True, stop=False)
                    nc.tensor.matmul(out=z1_ps[:], lhsT=fw1b[:], rhs=ptB[:],
                                     start=False, stop=True)
                    z1 = work.tile([HID, P], dt.bfloat16, tag="z1s")
                    nc.scalar.activation(z1[:], z1_ps[:], AF.Relu, bias=fb1c[:])

                    z2_ps = head_ps.tile([1, P], dt.float32, tag="z2")
                    nc.tensor.matmul(out=z2_ps[:], lhsT=fw2[:], rhs=z1[:],
                                     start=True, stop=True)
                    z2 = work.tile([1, P], dt.float32, tag="z2s")
                    nc.scalar.activation(z2[:], z2_ps[:], AF.Copy, bias=float(fb2))
                    # softmax over a width-1 axis == 1.0 for finite logits
                    outs = work.tile([1, P], dt.float32, tag="outs")
                    nc.vector.tensor_tensor(out=outs[:], in0=z2[:], in1=z2[:],
                                            op=OP.is_equal)
                    nc.sync.dma_start(out=out_d[:], in_=outs[:])

    nc.compile()
    return nc


# ---------------------------------------------------------------- run

_CACHE = {}


def _get_nc(st, fb2):
    import os as _os
    key = (st.key(), fb2, _os.environ.get('GCN_DEBUG_LAYERS', '3'))
    if key not in _CACHE:
        _CACHE[key] = build_bass(st, fb2)
    return _CACHE[key]


def make_in_maps(inputs):
    st, cores, wts, fb2 = build_host_data(inputs)
    in_maps = [dict(**cores[c], **wts) for c in range(NCORES)]
    return st, in_maps, fb2


LAST_RESULTS = None


def kernel(**inputs):
    global LAST_RESULTS
    st, in_maps, fb2 = make_in_maps(inputs)
    nc = _get_nc(st, fb2)
    from concourse.bass_utils import run_bass_kernel_spmd
    res = run_bass_kernel_spmd(nc, in_maps, core_ids=list(range(NCORES)))
    LAST_RESULTS = res
    out = np.asarray(res.results[0]["out"]).reshape(P)[:G]
    return out.reshape(G, 1).astype(np.float32)
